# revision 11
# baseline (speedup 1.0000x reference)
"""GRU kernel for Trainium2, 8 NeuronCores.

Strategy: tensor-parallel over hidden_size (8 slices of 128).
- Input projections x @ W_i_*^T computed per-core for its H-slice (bf16 PE).
- Recurrence: per step, each core computes its h-slice via 24 bf16 matmuls
  (lhsT = W_h^T tiles, rhs = transposed state hT [128,64]), gate elementwise
  on DVE/ACT, then broadcasts its new h-slice into every peer's SBUF ring
  via remote_dma_broadcast (SWDGE), with remote semaphores gating step t+1.

Raw bacc program (manual semaphores), fully unrolled over T.
"""
import sys
import os

sys.path.insert(0, "/opt/trn_rl_repo")

import numpy as np
import ml_dtypes

import concourse.bass as bass
import concourse.bacc as bacc
import concourse.mybir as mybir
from concourse import bass_utils

F32 = mybir.dt.float32
BF16 = mybir.dt.bfloat16
AF = mybir.ActivationFunctionType
ds = bass.ds

B = 64          # batch
D = 512         # input size
H = 1024        # hidden size
NC = 8          # cores
HS = H // NC    # per-core hidden slice = 128
T_FULL = 512


def build(T=T_FULL, check_groups=True):
    NB = T * B // 512          # projection N-chunks of 512 (t-major cols)
    nc = bacc.Bacc("TRN2", target_bir_lowering=False, debug=False, num_devices=NC)

    # ---- I/O ----
    xt_in = nc.dram_tensor("xt_in", [NB, 128, 4, 512], BF16, kind="ExternalInput")
    wi_in = nc.dram_tensor("wi_in", [128, 1536], BF16, kind="ExternalInput")
    wh_in = nc.dram_tensor("wh_in", [128, 3072], BF16, kind="ExternalInput")
    bias_in = nc.dram_tensor("bias_in", [128, 4], F32, kind="ExternalInput")
    out_d = nc.dram_tensor("out", [T, 128, B], F32, kind="ExternalOutput")
    xproj = nc.dram_tensor("xproj", [128, T, 3, B], F32, kind="Internal")

    # ---- SBUF ----
    wi_sb = nc.alloc_sbuf_tensor("wi_sb", [128, 1536], BF16)
    wh_sb = nc.alloc_sbuf_tensor("wh_sb", [128, 3072], BF16)
    bias_sb = nc.alloc_sbuf_tensor("bias_sb", [128, 4], F32)
    xchunk = [nc.alloc_sbuf_tensor(f"xchunk{i}", [128, 2048], BF16) for i in range(2)]
    pj_out = [nc.alloc_sbuf_tensor(f"pjout{i}", [128, 512], F32) for i in range(6)]
    ring = nc.alloc_sbuf_tensor("ring", [128, 4 * 512], BF16)   # 4 slots x [128, 8*64]
    xt = [nc.alloc_sbuf_tensor(f"xt{i}", [128, 192], F32) for i in range(4)]
    pre_rz = [nc.alloc_sbuf_tensor(f"prerz{i}", [128, 128], F32) for i in range(2)]
    rz = [nc.alloc_sbuf_tensor(f"rz{i}", [128, 128], F32) for i in range(2)]
    hn = [nc.alloc_sbuf_tensor(f"hn{i}", [128, 64], F32) for i in range(2)]
    t2 = [nc.alloc_sbuf_tensor(f"t2_{i}", [128, 64], F32) for i in range(2)]
    ntile = [nc.alloc_sbuf_tensor(f"ntile{i}", [128, 64], F32) for i in range(2)]
    tmp1 = nc.alloc_sbuf_tensor("tmp1", [128, 64], F32)
    tmp2 = nc.alloc_sbuf_tensor("tmp2", [128, 64], F32)
    dmn = nc.alloc_sbuf_tensor("dmn", [128, 64], F32)
    hprev = [nc.alloc_sbuf_tensor(f"hprev{i}", [128, 64], F32) for i in range(2)]
    hbf = [nc.alloc_sbuf_tensor(f"hbf{i}", [128, 64], BF16) for i in range(2)]

    # PSUM: 6 banks [128,512] f32; recurrence reuses banks 0 and 3 (cols 0:192)
    pj_ps = [nc.alloc_psum_tensor(f"pjps{i}", [128, 512], F32) for i in range(6)]

    # ---- semaphores ----
    S = lambda n: nc.alloc_semaphore(n)
    s_init, s_wload = S("init"), S("wload")
    s_pjx, s_pjmm, s_pjact, s_pjev = S("pjx"), S("pjmm"), S("pjact"), S("pjev")
    s_rsem = [S(f"rsem{i}") for i in range(4)]
    s_lsem = [S(f"lsem{i}") for i in range(2)]
    s_xt, s_mm = S("sxt"), S("smm")
    s_d1, s_psfree, s_pret, s_hnew, s_hbf = S("d1"), S("psf"), S("pret"), S("hnew"), S("hbf")
    s_dvec, s_dved, s_dvee = S("dvec"), S("dved"), S("dvee")
    s_sig, s_tanh, s_outd = S("sig"), S("tanh"), S("outd")

    PE, VE, SE, GP, SP = nc.tensor, nc.vector, nc.scalar, nc.gpsimd, nc.sync

    # =========================================================
    # init: weight loads (SP), memsets (GP)
    # =========================================================
    SP.dma_start(wi_sb.ap()[:], wi_in.ap()[:]).then_inc(s_wload, 16)
    SP.dma_start(wh_sb.ap()[:], wh_in.ap()[:]).then_inc(s_wload, 16)
    SP.dma_start(bias_sb.ap()[:], bias_in.ap()[:]).then_inc(s_wload, 16)
    GP.memset(ring.ap()[:, 0:512], 0.0).then_inc(s_init, 1)
    GP.memset(hprev[0].ap()[:], 0.0).then_inc(s_init, 1)

    pid = GP.partition_id()   # RuntimeValue on Pool

    # =========================================================
    # Projection phase: xproj[:, t, g, b] = (x @ W_i_g^T)[slice] + bias_g
    # =========================================================
    # PE: per chunk nb: 3 gates x 4 K-chunks matmuls into pj_ps[(nb%2)*3+g]
    for nb in range(NB):
        if nb == 0:
            PE.wait_ge(s_wload, 48)
        PE.wait_ge(s_pjx, 16 * (nb + 1))
        if nb >= 2:
            PE.wait_ge(s_pjact, 3 * (nb - 2) + 3)
        for g in range(3):
            for k in range(4):
                mm = PE.matmul(
                    pj_ps[(nb % 2) * 3 + g].ap()[:],
                    wi_sb.ap()[:, k * 384 + g * 128: k * 384 + (g + 1) * 128],
                    xchunk[nb % 2].ap()[:, k * 512:(k + 1) * 512],
                    start=(k == 0), stop=(k == 3),
                )
                if k == 3:
                    mm.then_inc(s_pjmm, 1)
    # ACT: evict with bias
    for nb in range(NB):
        for g in range(3):
            if nb >= 2 and g == 0:
                SE.wait_ge(s_pjev, 16 * (3 * (nb - 2) + 3))
            a = SE.activation(
                pj_out[(nb % 2) * 3 + g].ap()[:],
                pj_ps[(nb % 2) * 3 + g].ap()[:],
                AF.Identity,
                bias=bias_sb.ap()[:, g:g + 1],
            )
            a._wait_ge(s_pjmm, 3 * nb + g + 1)
            a.then_inc(s_pjact, 1)
    # SP: x chunk loads + evictions
    for nb in range(NB):
        if nb >= 1:
            SP.wait_ge(s_pjx, 16 * nb)          # chain: in-order completion
        if nb >= 2:
            SP.wait_ge(s_pjmm, 3 * (nb - 2) + 3)
        ld = SP.dma_start(xchunk[nb % 2].ap()[:], xt_in.ap()[nb])
        ld.then_inc(s_pjx, 16)
        for g in range(3):
            if 3 * nb + g >= 1:
                SP.wait_ge(s_pjev, 16 * (3 * nb + g))   # chain evicts
            # pj_out [128, 512] = (8 t) x (64 b) -> xproj[:, nb*8:(nb+1)*8, g, :]
            ev = SP.dma_start(
                xproj.ap()[:, nb * 8:(nb + 1) * 8, g, :],
                pj_out[(nb % 2) * 3 + g].ap()[:].rearrange("p (t b) -> p t b", t=8),
            )
            ev._wait_ge(s_pjact, 3 * nb + g + 1)
            ev.then_inc(s_pjev, 16)

    # =========================================================
    # Recurrence: steps t = 1..T
    # =========================================================
    # PE stream
    for t in range(1, T + 1):
        if t >= 2:
            PE.wait_ge(s_rsem[(t - 1) % 4], 16 * (((t - 1) + 3) // 4))
        else:
            PE.wait_ge(s_init, 2)
        if t >= 3:
            PE.wait_ge(s_psfree, t - 2)
        if t in (1, 2):
            PE.wait_ge(s_pjact, 3 * NB)
        for k in range(8):
            for g in range(3):
                mm = PE.matmul(
                    pj_ps[(t % 2) * 3 + g].ap()[:, 0:64],
                    wh_sb.ap()[:, k * 384 + g * 128: k * 384 + (g + 1) * 128],
                    ring.ap()[:, ((t - 1) % 4) * 512 + k * 64: ((t - 1) % 4) * 512 + (k + 1) * 64],
                    start=(k == 0), stop=(k == 7),
                )
                if k == 7 and g == 2:
                    mm.then_inc(s_mm, 1)

    # DVE stream
    for t in range(1, T + 1):
        ps_r = pj_ps[(t % 2) * 3 + 0].ap()[:, 0:64]
        ps_z = pj_ps[(t % 2) * 3 + 1].ap()[:, 0:64]
        ps_n = pj_ps[(t % 2) * 3 + 2].ap()[:, 0:64]
        i2, i4 = t % 2, t % 4
        VE.wait_ge(s_xt, 16 * t)
        d1a = VE.tensor_add(pre_rz[i2].ap()[:, 0:64], ps_r, xt[i4].ap()[:, 0:64])
        d1a._wait_ge(s_mm, t)
        d1a.then_inc(s_d1, 1)
        d1 = VE.tensor_add(pre_rz[i2].ap()[:, 64:128], ps_z, xt[i4].ap()[:, 64:128])
        d1.then_inc(s_d1, 1)
        d2 = VE.tensor_scalar_add(hn[i2].ap()[:], ps_n, bias_sb.ap()[:, 3:4])
        d2.then_inc(s_psfree, 1)
        VE.wait_ge(s_psfree, t)
        d3 = VE.tensor_mul(tmp1.ap()[:], rz[i2].ap()[:, 0:64], hn[i2].ap()[:])
        d3._wait_ge(s_sig, t)
        d3.then_inc(s_dvec, 1)
        d4 = VE.tensor_add(t2[i2].ap()[:], tmp1.ap()[:], xt[i4].ap()[:, 128:192])
        d4._wait_ge(s_dvec, t)
        d4.then_inc(s_pret, 1)
        d5 = VE.tensor_sub(dmn.ap()[:], hprev[(t - 1) % 2].ap()[:], ntile[i2].ap()[:])
        d5._wait_ge(s_tanh, t)
        d5.then_inc(s_dved, 1)
        d6 = VE.tensor_mul(tmp2.ap()[:], rz[i2].ap()[:, 64:128], dmn.ap()[:])
        d6._wait_ge(s_dved, t)
        d6.then_inc(s_dvee, 1)
        VE.wait_ge(s_dvee, t)
        if t >= 3:
            VE.wait_ge(s_outd, 16 * (t - 2))
        d7 = VE.tensor_add(hprev[i2].ap()[:], tmp2.ap()[:], ntile[i2].ap()[:])
        d7.then_inc(s_hnew, 1)
        if t >= 3:
            VE.wait_ge(s_lsem[t % 2], 16 * ((t - 1) // 2))
        d8 = VE.tensor_copy(hbf[i2].ap()[:], hprev[i2].ap()[:])
        d8._wait_ge(s_hnew, t)
        d8.then_inc(s_hbf, 1)

    # ACT stream (activations + output stores interleaved — in-order engine)
    for t in range(1, T + 1):
        i2 = t % 2
        a1 = SE.activation(rz[i2].ap()[:], pre_rz[i2].ap()[:], AF.Sigmoid)
        a1._wait_ge(s_d1, 2 * t)
        a1.then_inc(s_sig, 1)
        a2 = SE.activation(ntile[i2].ap()[:], t2[i2].ap()[:], AF.Tanh)
        a2._wait_ge(s_pret, t)
        a2.then_inc(s_tanh, 1)
        if t >= 2:
            SE.wait_ge(s_outd, 16 * (t - 1))    # chain stores
        st = SE.dma_start(out_d.ap()[t - 1], hprev[t % 2].ap()[:])
        st._wait_ge(s_hnew, t)
        st.then_inc(s_outd, 16)

    # GPSIMD stream: broadcast h_t into ring slot (t%4), col pid*64, on all cores
    rdests = [(0, d) for d in range(NC)]
    for t in range(1, T):
        GP.remote_dma_broadcast(
            ring.ap()[:, ds((t % 4) * 512 + pid * 64, 64)],
            hbf[t % 2].ap()[:],
            remote_sem=s_rsem[t % 4],
            local_sem=s_lsem[t % 2],
            rdests=rdests,
        )
        trg = GP.trigger_dma(1)
        trg._wait_ge(s_hbf, t)

    # SP stream: xt prefetch (depth 2)
    for t in range(1, T + 1):
        nb_need = min(NB, (t - 1) // 8 + 1)
        if t >= 2:
            SP.wait_ge(s_xt, 16 * (t - 1))      # chain xt loads
        if t >= 5:
            SP.wait_ge(s_pret, t - 4)
        ld = SP.dma_start(
            xt[t % 4].ap()[:].rearrange("p (g b) -> p g b", g=3),
            xproj.ap()[:, t - 1, :, :],
        )
        ld._wait_ge(s_pjev, 16 * 3 * nb_need)
        ld.then_inc(s_xt, 16)

    nc.compile()
    return nc


# =========================================================
# Host side
# =========================================================
_CACHE = {}


def _get_nc(T):
    if T not in _CACHE:
        _CACHE[T] = build(T)
    return _CACHE[T]


def _prep_inputs(x, W_i_r, b_i_r, W_h_r, b_h_r, W_i_z, b_i_z, W_h_z, b_h_z,
                 W_i_n, b_i_n, W_h_n, b_h_n, T):
    bf = ml_dtypes.bfloat16
    Bn = x.shape[0]
    # x [B, T, D] -> xT [D, T, B] -> chunks [NB, 128, 4, 512]
    xT = np.ascontiguousarray(np.transpose(np.asarray(x, np.float32), (2, 1, 0)))
    xT = xT.reshape(4, 128, T * Bn // 512, 512)        # [k, p, nb, n]
    xt_chunks = np.ascontiguousarray(xT.transpose(2, 1, 0, 3)).astype(bf)  # [nb,p,k,n]

    in_maps = []
    Wis = [np.asarray(w, np.float32) for w in (W_i_r, W_i_z, W_i_n)]
    Whs = [np.asarray(w, np.float32) for w in (W_h_r, W_h_z, W_h_n)]
    bis = [np.asarray(b, np.float32) for b in (b_i_r, b_i_z, b_i_n)]
    bhs = [np.asarray(b, np.float32) for b in (b_h_r, b_h_z, b_h_n)]
    for c in range(NC):
        sl = slice(c * HS, (c + 1) * HS)
        wi = np.empty((128, 1536), np.float32)
        wh = np.empty((128, 3072), np.float32)
        for k in range(4):
            for g in range(3):
                # lhsT tile [K=128 (D), M=128 (H slice)] = W_i_g[sl, kD].T
                wi[:, k * 384 + g * 128:k * 384 + (g + 1) * 128] = \
                    Wis[g][sl, k * 128:(k + 1) * 128].T
        for k in range(8):
            for g in range(3):
                wh[:, k * 384 + g * 128:k * 384 + (g + 1) * 128] = \
                    Whs[g][sl, k * 128:(k + 1) * 128].T
        bias = np.empty((128, 4), np.float32)
        bias[:, 0] = bis[0][sl] + bhs[0][sl]
        bias[:, 1] = bis[1][sl] + bhs[1][sl]
        bias[:, 2] = bis[2][sl]
        bias[:, 3] = bhs[2][sl]
        in_maps.append({
            "xt_in": xt_chunks,
            "wi_in": wi.astype(bf),
            "wh_in": wh.astype(bf),
            "bias_in": bias,
        })
    return in_maps


def kernel(x, W_i_r, b_i_r, W_h_r, b_h_r, W_i_z, b_i_z, W_h_z, b_h_z,
           W_i_n, b_i_n, W_h_n, b_h_n, _trace=False, _tmpdir=None):
    Bn, T, _ = x.shape
    nc = _get_nc(T)
    in_maps = _prep_inputs(x, W_i_r, b_i_r, W_h_r, b_h_r, W_i_z, b_i_z,
                           W_h_z, b_h_z, W_i_n, b_i_n, W_h_n, b_h_n, T)
    kw = {}
    if _trace:
        kw = dict(trace=True, tmpdir=_tmpdir)
    res = bass_utils.run_bass_kernel_spmd(nc, in_maps, core_ids=list(range(NC)), **kw)
    # per-core out [T, 128, B] -> h_seq [B, T, H]
    outs = np.stack([res.results[c]["out"] for c in range(NC)])   # [NC, T, 128, B]
    h_seq = np.ascontiguousarray(outs.transpose(3, 1, 0, 2)).reshape(Bn, T, H)
    kernel._last_exec_ns = res.exec_time_ns
    return h_seq
